# revision 53
# baseline (speedup 1.0000x reference)
"""BBoxScoreHead Trainium2 kernel (8-core data-parallel).

Strategy
--------
Data-parallel over batch: B=64 -> 8 samples per NeuronCore.

Per sample b the reference computes, for feat [C,H,W]:
  pooled[c]  = (1/area_b) * sum_{h,w} feat[c,h,w] * row_b[h] * col_b[w]
  global[c]  = (1/(H*W))  * sum_{h,w} feat[c,h,w]
where row_b/col_b are 0/1 interval masks derived from boxes (host-computable,
O(B*(H+W)) work), then a tiny 3-layer MLP on [pooled | global | lang].

feat is staged host-side as fp8 e4m3 in [h, b, w, c] layout (empirically
safe: quantization error at the final sigmoid output is ~3e-5, the gate is
2e-2).  fp8 halves HBM traffic vs bf16; under the all-8-cores HBM storm the
16 SDMA engines sustain ~15 GB/s each (~232 GB/s/core) regardless of
descriptor size, so feat streams per-sample (28.7 KB descriptors) to
minimize the first-compute latency.

Both reductions are TensorE matmuls contracting h (112 partitions) with
perf_mode=DoubleRow: fp8 pairs along the contract dim = (w-parity).  Each
matmul covers a w-QUAD (4 w's: pair p in {0,1} x free wo in {0,1}) with a
3-column stationary [ones | row*col(wo=0 w's) | row*col(wo=1 w's)], so a
sample is 28 matmuls of 512 output columns accumulated in PSUM f32.
Per-sample folds produce tg2/tp2 [8 x 256] result tiles (row = sample);
the tail is just 6 PE transposes + the tiny MLP.
"""

import sys

if "/opt/trn_rl_repo" not in sys.path:
    sys.path.insert(0, "/opt/trn_rl_repo")

import numpy as np

B, C, H, W = 64, 256, 112, 112
N_CORES = 8
BS = B // N_CORES          # samples per core
CH = 128                   # channel half
NQ = W // 4                # w-quads
LANG = 256
HID = 256
WPAD = 16                  # stationary pair-stride pad (elements)

_CACHE = {}


# ---------------------------------------------------------------- host masks
def _host_masks(boxes_xywh):
    """Replicates reference._boxes_xywh_to_clamped_xyxy + margin/mask logic
    in float32 numpy. Returns row [B,H], col [B,W], area [B] (float32)."""
    b = boxes_xywh.astype(np.float32)
    xc, yc, w, h = b[:, 0], b[:, 1], b[:, 2], b[:, 3]
    x1 = xc - w / 2.0
    y1 = yc - h / 2.0
    x2 = xc + w / 2.0
    y2 = yc + h / 2.0
    eps = 1e-6
    x1 = np.clip(x1, 0.0, 1.0)
    x2 = np.clip(x2, 0.0, 1.0)
    y1 = np.clip(y1, 0.0, 1.0)
    y2 = np.clip(y2, 0.0, 1.0)
    x_lo, x_hi = np.minimum(x1, x2), np.maximum(x1, x2)
    y_lo, y_hi = np.minimum(y1, y2), np.maximum(y1, y2)
    w = np.maximum(x_hi - x_lo, eps)
    h = np.maximum(y_hi - y_lo, eps)
    cx = (x_hi + x_lo) * 0.5
    cy = (y_hi + y_lo) * 0.5
    x1 = np.clip(cx - w * 0.5, 0.0, 1.0)
    x2 = np.clip(cx + w * 0.5, 0.0, 1.0)
    y1 = np.clip(cy - h * 0.5, 0.0, 1.0)
    y2 = np.clip(cy + h * 0.5, 0.0, 1.0)

    bw = np.maximum(x2 - x1, 1e-4)
    bh = np.maximum(y2 - y1, 1e-4)
    margin = np.clip(np.sqrt(bw * bw + bh * bh) * 0.25, 0.02, 0.18)
    mx1 = np.clip(x1 - margin, 0.0, 1.0)
    my1 = np.clip(y1 - margin, 0.0, 1.0)
    mx2 = np.clip(x2 + margin, 0.0, 1.0)
    my2 = np.clip(y2 + margin, 0.0, 1.0)

    ys = np.linspace(0.0, 1.0, H).astype(np.float32)
    xs = np.linspace(0.0, 1.0, W).astype(np.float32)
    row = ((ys[None, :] >= my1[:, None]) & (ys[None, :] <= my2[:, None]))
    col = ((xs[None, :] >= mx1[:, None]) & (xs[None, :] <= mx2[:, None]))
    row = row.astype(np.float32)
    col = col.astype(np.float32)
    area = np.maximum(row.sum(axis=1) * col.sum(axis=1), 1.0).astype(np.float32)
    return row, col, area


def _build_wm(row, col):
    """DoubleRow stationary mask-weights, [H, bs, NQ, 2, WPAD] fp8.
    w = 4q + 2*wo + p.  m=0: ones (global); m=1: row*col[4q+p] (wo=0
    masked); m=2: row*col[4q+2+p] (wo=1 masked).  0/1 exact in fp8."""
    import ml_dtypes
    bs = row.shape[0]
    wm = np.zeros((H, bs, NQ, 2, WPAD), dtype=np.float32)
    wm[:, :, :, :, 0] = 1.0
    cq = col.reshape(bs, NQ, 2, 2)                         # [bs, q, wo, p]
    wm[:, :, :, :, 1] = row.T[:, :, None, None] * cq[None, :, :, 0, :]
    wm[:, :, :, :, 2] = row.T[:, :, None, None] * cq[None, :, :, 1, :]
    return wm.astype(ml_dtypes.float8_e4m3)


# ---------------------------------------------------------------- bass build
def _build_nc():
    import concourse.tile as tile
    from concourse import bacc, mybir

    f32 = mybir.dt.float32
    bf16 = mybir.dt.bfloat16
    fp8 = mybir.dt.float8e4
    Ident = mybir.ActivationFunctionType.Identity
    Relu = mybir.ActivationFunctionType.Relu
    Sigmoid = mybir.ActivationFunctionType.Sigmoid
    DR = mybir.MatmulPerfMode.DoubleRow

    nc = bacc.Bacc("TRN2", target_bir_lowering=False, debug=False,
                   num_devices=N_CORES)

    # [h, b, w, c] fp8 layout: feat[:, b] is one contiguous 28672-byte run
    # per partition.
    feat = nc.dram_tensor("feat", [H, BS, W, C], fp8, kind="ExternalInput")
    ident = nc.dram_tensor("ident", [32, 32], f32, kind="ExternalInput")
    wm = nc.dram_tensor("wm", [H, BS, NQ, 2, WPAD], fp8,
                        kind="ExternalInput")
    lang = nc.dram_tensor("lang", [BS, LANG], f32, kind="ExternalInput")
    # per-acc-row scales: [1/(H*W), 1/area_b, 1/area_b]
    psc3 = nc.dram_tensor("psc3", [3, BS], f32, kind="ExternalInput")
    w1t = nc.dram_tensor("w1t", [128, 6 * HID], bf16, kind="ExternalInput")
    w2t = nc.dram_tensor("w2t", [128, 4 * 128], bf16, kind="ExternalInput")
    w3t = nc.dram_tensor("w3t", [128, 2], bf16, kind="ExternalInput")
    b1 = nc.dram_tensor("b1", [128, 2], f32, kind="ExternalInput")
    b2 = nc.dram_tensor("b2", [128, 2], f32, kind="ExternalInput")
    b3 = nc.dram_tensor("b3", [1, 1], f32, kind="ExternalInput")
    out = nc.dram_tensor("out", [1, BS], f32, kind="ExternalOutput")

    with tile.TileContext(nc) as tc:
        with (
            tc.tile_pool(name="ft", bufs=4) as ftp,
            tc.tile_pool(name="const", bufs=1) as cp,
            tc.tile_pool(name="stage", bufs=3) as stp,
            tc.tile_pool(name="small", bufs=1) as sp,
            tc.tile_pool(name="acca", bufs=2, space="PSUM") as pa,
            tc.tile_pool(name="accb", bufs=2, space="PSUM") as pb,
            tc.tile_pool(name="mlp", bufs=1, space="PSUM") as mpp,
        ):
            # ---- all constants go on the scalar HWDGE ring; the sync ring
            # carries only the 8 per-sample feat streams (emitted in the
            # sample loop below, pipelined 4 deep by the pool).  Sample 0's
            # mask slice loads first so the first matmul isn't gated on the
            # whole wm tensor.
            # wm as ONE dma (slicing it made 448-byte descriptors, below the
            # 512B line-rate floor -- it took ~11us and gated the first mm)
            wm_sb = cp.tile([H, BS, NQ, 2, WPAD], fp8)
            nc.scalar.dma_start(wm_sb[:], wm[:])
            psc3_sb = cp.tile([3, BS], f32)
            nc.scalar.dma_start(psc3_sb[:], psc3[:])
            id_sb = cp.tile([32, 32], f32)
            nc.scalar.dma_start(id_sb[:], ident[:])
            # warm the scalar-engine activation tables off the critical path
            warm = sp.tile([1, 1], f32, tag="warm")
            nc.scalar.activation(warm[:], psc3_sb[0:1, 0:1], Ident)
            nc.scalar.activation(warm[:], psc3_sb[0:1, 0:1], Relu)
            nc.scalar.activation(warm[:], psc3_sb[0:1, 0:1], Sigmoid)
            # remaining consts are emitted inside the sample loop (below) so
            # their scalar-ring slots sit behind sample 0's fold in FIFO
            # order and don't steal SDMA-engine time from sample 0's feat.
            w1t_sb = cp.tile([128, 6 * HID], bf16)
            w2t_sb = cp.tile([128, 4 * 128], bf16)
            w3t_sb = cp.tile([128, 2], bf16)
            b1_sb = cp.tile([128, 2], f32)
            b2_sb = cp.tile([128, 2], f32)
            b3_sb = cp.tile([1, 1], f32)
            lt = cp.tile([BS, LANG], f32)
            # combined.T staged in SBUF as bf16 for the single-pass MLP
            ct = cp.tile([128, 48], bf16)

            # ---- stage 1: masked + global pooling via fp8 DoubleRow matmuls
            # Each sample streams as two w-halves (separate tiles) so its
            # first 14 matmuls overlap the second half's DMA.
            HQ = NQ // 2
            folds = []
            prev_fth = None
            for b in range(BS):
                # dual-bank accumulation: even q -> acc_a, odd q -> acc_b,
                # so consecutive matmuls hit different PSUM banks and can
                # pipeline instead of serializing on the bank RMW.
                acc_a = pa.tile([3, 2 * C], f32, tag="acca")
                acc_b = pb.tile([3, 2 * C], f32, tag="accb")
                # first/last sample stream in quarters: sample 0 so the
                # first matmul isn't gated on a full half-sample DMA, sample
                # 7 so only a quarter's matmuls trail the last DMA packet
                nsub = 2 if b in (0, BS - 1) else 1
                for half in range(2):
                    fth = ftp.tile([H, W // 2, C], fp8, tag=f"ft{half}")
                    if prev_fth is not None:
                        # keep-alive: the PE idles ~3-4us per half waiting
                        # on the DMA, HAM re-throttles it to ~1.2GHz, and
                        # the cold serial matmuls then run slower than the
                        # DMA cadence (a limit cycle).  Shorten every gap
                        # below the HAM window with matmuls whose stationary
                        # is the wm ZERO padding (cols 4:7): they accumulate
                        # exact zeros, so the real sums are untouched.
                        pmv = prev_fth[:].rearrange(
                            "h (q wo p) c -> h q p wo c", wo=2, p=2)
                        for dq in range(3):
                            nc.tensor.matmul(
                                acc_a[:], wm_sb[:, 0, dq, :, 4:7],
                                pmv[:, dq], start=False, stop=False,
                                perf_mode=DR, skip_group_check=True)
                    for su in range(nsub):
                        nc.sync.dma_start(
                            fth[:, su * (56 // nsub):(su + 1) * (56 // nsub), :],
                            feat[:, b, half * 56 + su * (56 // nsub):
                                 half * 56 + (su + 1) * (56 // nsub), :])
                    # [h, q, p, wo, c]: w_local = 4q + 2wo + p; dim p is the
                    # DoubleRow contract pair, (wo, c) the 512 moving columns.
                    mv = fth[:].rearrange("h (q wo p) c -> h q p wo c",
                                          wo=2, p=2)
                    for q in range(HQ):
                        qg = half * HQ + q
                        acc = acc_a if qg % 2 == 0 else acc_b
                        nc.tensor.matmul(
                            acc[:],
                            wm_sb[:, b, qg, :, 0:3],
                            mv[:, q],
                            start=(qg < 2),
                            stop=(qg >= NQ - 2),
                            perf_mode=DR,
                            skip_group_check=(b >= BS - 2),
                        )
                    prev_fth = fth
                # acc = [3 rows, (wo, c)]: row0 global, row1 wo=0 masked,
                # row2 wo=1 masked.  PSUM->SBUF copy with the per-row scale
                # (1/(H*W) or 1/area_b) fused in.
                # DVE may read only one PSUM operand: stage bank B first
                sbb = stp.tile([3, 2 * C], f32, tag="sbb")
                nc.vector.tensor_copy(sbb[:], acc_b[:])
                sallr = stp.tile([3, 2 * C], f32, tag="sallr")
                nc.vector.tensor_add(sallr[:], acc_a[:], sbb[:])
                sall = stp.tile([3, 2 * C], f32, tag="sall")
                nc.scalar.activation(sall[:], sallr[:], Ident,
                                     scale=psc3_sb[0:3, b:b + 1])
                sall_v = sall[:].rearrange("p (wo c) -> p wo c", wo=2)
                sg = stp.tile([1, C], f32, tag="sg")
                nc.vector.tensor_add(sg[0:1, :], sall_v[0:1, 0, :],
                                     sall_v[0:1, 1, :])
                # rows 1,2 -> partition 0 (DMA crosses partitions)
                rowp = stp.tile([1, 4 * C], f32, tag="rowp")
                nc.scalar.dma_start(rowp[:], sall[1:3, :])
                rowp_v = rowp[:].rearrange("p (m wo c) -> p m wo c",
                                           m=2, wo=2)
                sm = stp.tile([1, C], f32, tag="sm")
                nc.vector.tensor_add(sm[0:1, :], rowp_v[0:1, 0, 0, :],
                                     rowp_v[0:1, 1, 1, :])
                # transpose into CT columns (pooled -> cols k*8+b, global ->
                # 16+k*8+b), delayed one sample so the PE queue never stalls
                # on a fold still in flight.
                if b == 0:
                    ctp = mpp.tile([128, 48], f32, tag="ctp")
                    # big consts ride behind sample 0's fold in FIFO order
                    nc.scalar.dma_start(w1t_sb[:], w1t[:])
                    nc.scalar.dma_start(w2t_sb[:], w2t[:])
                    nc.scalar.dma_start(w3t_sb[:], w3t[:])
                    nc.scalar.dma_start(b1_sb[:], b1[:])
                    nc.scalar.dma_start(b2_sb[:], b2[:])
                    nc.scalar.dma_start(b3_sb[:], b3[:])
                    nc.scalar.dma_start(lt[:], lang[:])
                if b == 1:
                    # lang transposes + their CT cast ride mid-stream (PE
                    # and DVE have slack); only pooled/global cols remain
                    # on the tail.
                    for k in range(2):
                        nc.tensor.transpose(
                            ctp[:, 32 + k * 8:32 + k * 8 + 8],
                            lt[:, k * 128:(k + 1) * 128],
                            id_sb[0:BS, 0:BS])
                    nc.vector.tensor_copy(ct[:, 32:48], ctp[:, 32:48])
                folds.append((b, sm, sg))
                for fb, fsm, fsg in folds[-2:-1] if b < BS - 1 else folds[-2:]:
                    for k in range(2):
                        nc.tensor.transpose(
                            ctp[:, k * 8 + fb:k * 8 + fb + 1],
                            fsm[0:1, k * CH:(k + 1) * CH], id_sb[0:1, 0:1])
                        nc.tensor.transpose(
                            ctp[:, 16 + k * 8 + fb:16 + k * 8 + fb + 1],
                            fsg[0:1, k * CH:(k + 1) * CH], id_sb[0:1, 0:1])

            nc.vector.tensor_copy(ct[:, 0:32], ctp[:, 0:32])

            rhs_k = [ct[:, 8 * k:8 * k + 8] for k in range(6)]

            # ---- layer 1: 768 -> 256, relu
            h1 = []
            for m2 in range(2):
                hp = mpp.tile([128, BS], f32, tag="h1p")
                for k in range(6):
                    nc.tensor.matmul(
                        hp[:],
                        w1t_sb[:, k * HID + m2 * 128:k * HID + m2 * 128 + 128],
                        rhs_k[k],
                        start=(k == 0), stop=(k == 5))
                ht = sp.tile([128, BS], bf16, tag=f"h1_{m2}")
                nc.scalar.activation(ht[:], hp[:], Relu,
                                     bias=b1_sb[:, m2:m2 + 1])
                h1.append(ht)

            # ---- layer 2: 256 -> 256, relu
            h2 = []
            for m2 in range(2):
                hp = mpp.tile([128, BS], f32, tag="h2p")
                for kc in range(2):
                    nc.tensor.matmul(
                        hp[:],
                        w2t_sb[:, (kc * 2 + m2) * 128:(kc * 2 + m2) * 128 + 128],
                        h1[kc][:],
                        start=(kc == 0), stop=(kc == 1))
                ht = sp.tile([128, BS], bf16, tag=f"h2_{m2}")
                nc.scalar.activation(ht[:], hp[:], Relu,
                                     bias=b2_sb[:, m2:m2 + 1])
                h2.append(ht)

            # ---- layer 3: 256 -> 1, sigmoid
            s3 = mpp.tile([1, BS], f32, tag="s3")
            for kc in range(2):
                nc.tensor.matmul(s3[:], w3t_sb[:, kc:kc + 1], h2[kc][:],
                                 start=(kc == 0), stop=(kc == 1))
            res = sp.tile([1, BS], f32, tag="res")
            nc.scalar.activation(res[:], s3[:], Sigmoid, bias=b3_sb[0:1, 0:1])
            nc.sync.dma_start(out[:], res[:])

    nc.compile()
    return nc


# ----------------------------------------------------------------- entry
def _prepare_in_maps(feat, lang_vec, boxes_xywh, w1, b1, w2, b2, w3, b3):
    import ml_dtypes

    row, col, area = _host_masks(boxes_xywh)

    w1t_arr = np.ascontiguousarray(
        w1.astype(np.float32).T.reshape(6, 128, HID)
        .transpose(1, 0, 2).reshape(128, 6 * HID)).astype(ml_dtypes.bfloat16)
    w2t_arr = np.ascontiguousarray(
        w2.astype(np.float32).T.reshape(2, 128, 2, 128)
        .transpose(1, 0, 2, 3).reshape(128, 4 * 128)).astype(ml_dtypes.bfloat16)
    w3t_arr = np.ascontiguousarray(
        w3.astype(np.float32).T.reshape(2, 128).T
        ).astype(ml_dtypes.bfloat16)                        # [128, 2]
    b1_arr = np.ascontiguousarray(b1.astype(np.float32).reshape(2, 128).T)
    b2_arr = np.ascontiguousarray(b2.astype(np.float32).reshape(2, 128).T)
    b3_arr = b3.astype(np.float32).reshape(1, 1)

    feat = feat.astype(np.float32)
    lang_vec = np.ascontiguousarray(lang_vec.astype(np.float32))

    in_maps = []
    for i in range(N_CORES):
        s = slice(i * BS, (i + 1) * BS)
        wm = _build_wm(row[s], col[s])
        in_maps.append({
            # [h, b, w, c] fp8 staging (see module docstring)
            "feat": feat[s].transpose(2, 0, 3, 1)
                    .astype(ml_dtypes.float8_e4m3),
            "wm": np.ascontiguousarray(wm),
            "psc3": np.stack([np.full(BS, 1.0 / (H * W), np.float32),
                              (1.0 / area[s]).astype(np.float32),
                              (1.0 / area[s]).astype(np.float32)]),
            "lang": lang_vec[s],
            "ident": np.eye(32, dtype=np.float32),
            "w1t": w1t_arr, "w2t": w2t_arr, "w3t": w3t_arr,
            "b1": b1_arr, "b2": b2_arr, "b3": b3_arr,
        })
    return in_maps


def kernel(feat, lang_vec, boxes_xywh, w1, b1, w2, b2, w3, b3,
           _trace=False):
    from concourse.bass_utils import run_bass_kernel_spmd

    if "nc" not in _CACHE:
        _CACHE["nc"] = _build_nc()
    nc = _CACHE["nc"]

    args = [np.asarray(a) for a in
            (feat, lang_vec, boxes_xywh, w1, b1, w2, b2, w3, b3)]
    in_maps = _prepare_in_maps(*args)
    res = None
    for attempt in range(2):
        try:
            res = run_bass_kernel_spmd(nc, in_maps,
                                       core_ids=list(range(N_CORES)),
                                       trace=_trace)
            break
        except Exception:
            if attempt == 1:
                raise
    out = np.concatenate([res.results[i]["out"].reshape(BS, 1)
                          for i in range(N_CORES)], axis=0)
    _CACHE["last_exec_time_ns"] = res.exec_time_ns
    return out.astype(np.float32)


# revision 56
# speedup vs baseline: 1.0063x; 1.0063x over previous
"""BBoxScoreHead Trainium2 kernel (8-core data-parallel).

Strategy
--------
Data-parallel over batch: B=64 -> 8 samples per NeuronCore.

Per sample b the reference computes, for feat [C,H,W]:
  pooled[c]  = (1/area_b) * sum_{h,w} feat[c,h,w] * row_b[h] * col_b[w]
  global[c]  = (1/(H*W))  * sum_{h,w} feat[c,h,w]
where row_b/col_b are 0/1 interval masks derived from boxes (host-computable,
O(B*(H+W)) work), then a tiny 3-layer MLP on [pooled | global | lang].

feat is staged host-side as fp8 e4m3 in [h, b, w, c] layout (empirically
safe: quantization error at the final sigmoid output is ~3e-5, the gate is
2e-2).  fp8 halves HBM traffic vs bf16; under the all-8-cores HBM storm the
16 SDMA engines sustain ~15 GB/s each (~232 GB/s/core) regardless of
descriptor size, so feat streams per-sample (28.7 KB descriptors) to
minimize the first-compute latency.

Both reductions are TensorE matmuls contracting h (112 partitions) with
perf_mode=DoubleRow: fp8 pairs along the contract dim = (w-parity).  Each
matmul covers a w-QUAD (4 w's: pair p in {0,1} x free wo in {0,1}) with a
3-column stationary [ones | row*col(wo=0 w's) | row*col(wo=1 w's)], so a
sample is 28 matmuls of 512 output columns accumulated in PSUM f32.
Per-sample folds produce tg2/tp2 [8 x 256] result tiles (row = sample);
the tail is just 6 PE transposes + the tiny MLP.
"""

import sys

if "/opt/trn_rl_repo" not in sys.path:
    sys.path.insert(0, "/opt/trn_rl_repo")

import numpy as np

B, C, H, W = 64, 256, 112, 112
N_CORES = 8
BS = B // N_CORES          # samples per core
CH = 128                   # channel half
NQ = W // 4                # w-quads
LANG = 256
HID = 256
WPAD = 16                  # stationary pair-stride pad (elements)

_CACHE = {}


# ---------------------------------------------------------------- host masks
def _host_masks(boxes_xywh):
    """Replicates reference._boxes_xywh_to_clamped_xyxy + margin/mask logic
    in float32 numpy. Returns row [B,H], col [B,W], area [B] (float32)."""
    b = boxes_xywh.astype(np.float32)
    xc, yc, w, h = b[:, 0], b[:, 1], b[:, 2], b[:, 3]
    x1 = xc - w / 2.0
    y1 = yc - h / 2.0
    x2 = xc + w / 2.0
    y2 = yc + h / 2.0
    eps = 1e-6
    x1 = np.clip(x1, 0.0, 1.0)
    x2 = np.clip(x2, 0.0, 1.0)
    y1 = np.clip(y1, 0.0, 1.0)
    y2 = np.clip(y2, 0.0, 1.0)
    x_lo, x_hi = np.minimum(x1, x2), np.maximum(x1, x2)
    y_lo, y_hi = np.minimum(y1, y2), np.maximum(y1, y2)
    w = np.maximum(x_hi - x_lo, eps)
    h = np.maximum(y_hi - y_lo, eps)
    cx = (x_hi + x_lo) * 0.5
    cy = (y_hi + y_lo) * 0.5
    x1 = np.clip(cx - w * 0.5, 0.0, 1.0)
    x2 = np.clip(cx + w * 0.5, 0.0, 1.0)
    y1 = np.clip(cy - h * 0.5, 0.0, 1.0)
    y2 = np.clip(cy + h * 0.5, 0.0, 1.0)

    bw = np.maximum(x2 - x1, 1e-4)
    bh = np.maximum(y2 - y1, 1e-4)
    margin = np.clip(np.sqrt(bw * bw + bh * bh) * 0.25, 0.02, 0.18)
    mx1 = np.clip(x1 - margin, 0.0, 1.0)
    my1 = np.clip(y1 - margin, 0.0, 1.0)
    mx2 = np.clip(x2 + margin, 0.0, 1.0)
    my2 = np.clip(y2 + margin, 0.0, 1.0)

    ys = np.linspace(0.0, 1.0, H).astype(np.float32)
    xs = np.linspace(0.0, 1.0, W).astype(np.float32)
    row = ((ys[None, :] >= my1[:, None]) & (ys[None, :] <= my2[:, None]))
    col = ((xs[None, :] >= mx1[:, None]) & (xs[None, :] <= mx2[:, None]))
    row = row.astype(np.float32)
    col = col.astype(np.float32)
    area = np.maximum(row.sum(axis=1) * col.sum(axis=1), 1.0).astype(np.float32)
    return row, col, area


def _build_wm(row, col):
    """DoubleRow stationary mask-weights, [H, bs, NQ, 2, WPAD] fp8.
    w = 4q + 2*wo + p.  m=0: ones (global); m=1: row*col[4q+p] (wo=0
    masked); m=2: row*col[4q+2+p] (wo=1 masked).  0/1 exact in fp8."""
    import ml_dtypes
    bs = row.shape[0]
    wm = np.zeros((H, bs, NQ, 2, WPAD), dtype=np.float32)
    wm[:, :, :, :, 0] = 1.0
    cq = col.reshape(bs, NQ, 2, 2)                         # [bs, q, wo, p]
    wm[:, :, :, :, 1] = row.T[:, :, None, None] * cq[None, :, :, 0, :]
    wm[:, :, :, :, 2] = row.T[:, :, None, None] * cq[None, :, :, 1, :]
    return wm.astype(ml_dtypes.float8_e4m3)


# ---------------------------------------------------------------- bass build
def _build_nc():
    import concourse.tile as tile
    from concourse import bacc, mybir

    f32 = mybir.dt.float32
    bf16 = mybir.dt.bfloat16
    fp8 = mybir.dt.float8e4
    Ident = mybir.ActivationFunctionType.Identity
    Relu = mybir.ActivationFunctionType.Relu
    Sigmoid = mybir.ActivationFunctionType.Sigmoid
    DR = mybir.MatmulPerfMode.DoubleRow

    nc = bacc.Bacc("TRN2", target_bir_lowering=False, debug=False,
                   num_devices=N_CORES)

    # [h, b, w, c] fp8 layout: feat[:, b] is one contiguous 28672-byte run
    # per partition.
    feat = nc.dram_tensor("feat", [H, BS, W, C], fp8, kind="ExternalInput")
    ident = nc.dram_tensor("ident", [32, 32], f32, kind="ExternalInput")
    wm = nc.dram_tensor("wm", [H, BS, NQ, 2, WPAD], fp8,
                        kind="ExternalInput")
    lang = nc.dram_tensor("lang", [BS, LANG], f32, kind="ExternalInput")
    # per-acc-row scales: [1/(H*W), 1/area_b, 1/area_b]
    psc3 = nc.dram_tensor("psc3", [3, BS], f32, kind="ExternalInput")
    w1t = nc.dram_tensor("w1t", [128, 6 * HID], bf16, kind="ExternalInput")
    w2t = nc.dram_tensor("w2t", [128, 4 * 128], bf16, kind="ExternalInput")
    w3t = nc.dram_tensor("w3t", [128, 2], bf16, kind="ExternalInput")
    b1 = nc.dram_tensor("b1", [128, 2], f32, kind="ExternalInput")
    b2 = nc.dram_tensor("b2", [128, 2], f32, kind="ExternalInput")
    b3 = nc.dram_tensor("b3", [1, 1], f32, kind="ExternalInput")
    out = nc.dram_tensor("out", [1, BS], f32, kind="ExternalOutput")

    with tile.TileContext(nc) as tc:
        with (
            tc.tile_pool(name="ft", bufs=4) as ftp,
            tc.tile_pool(name="const", bufs=1) as cp,
            tc.tile_pool(name="stage", bufs=3) as stp,
            tc.tile_pool(name="small", bufs=1) as sp,
            tc.tile_pool(name="acc", bufs=4, space="PSUM") as pp,
            tc.tile_pool(name="mlp", bufs=1, space="PSUM") as mpp,
        ):
            # ---- all constants go on the scalar HWDGE ring; the sync ring
            # carries only the 8 per-sample feat streams (emitted in the
            # sample loop below, pipelined 4 deep by the pool).  Sample 0's
            # mask slice loads first so the first matmul isn't gated on the
            # whole wm tensor.
            # wm as ONE dma (slicing it made 448-byte descriptors, below the
            # 512B line-rate floor -- it took ~11us and gated the first mm)
            wm_sb = cp.tile([H, BS, NQ, 2, WPAD], fp8)
            nc.scalar.dma_start(wm_sb[:], wm[:])
            psc3_sb = cp.tile([3, BS], f32)
            nc.scalar.dma_start(psc3_sb[:], psc3[:])
            id_sb = cp.tile([32, 32], f32)
            nc.scalar.dma_start(id_sb[:], ident[:])
            # warm the scalar-engine activation tables off the critical path
            warm = sp.tile([1, 1], f32, tag="warm")
            nc.scalar.activation(warm[:], psc3_sb[0:1, 0:1], Ident)
            nc.scalar.activation(warm[:], psc3_sb[0:1, 0:1], Relu)
            nc.scalar.activation(warm[:], psc3_sb[0:1, 0:1], Sigmoid)
            # remaining consts are emitted inside the sample loop (below) so
            # their scalar-ring slots sit behind sample 0's fold in FIFO
            # order and don't steal SDMA-engine time from sample 0's feat.
            w1t_sb = cp.tile([128, 6 * HID], bf16)
            w2t_sb = cp.tile([128, 4 * 128], bf16)
            w3t_sb = cp.tile([128, 2], bf16)
            b1_sb = cp.tile([128, 2], f32)
            b2_sb = cp.tile([128, 2], f32)
            b3_sb = cp.tile([1, 1], f32)
            lt = cp.tile([BS, LANG], f32)
            # combined.T staged in SBUF as bf16 for the single-pass MLP
            ct = cp.tile([128, 48], bf16)

            # ---- stage 1: masked + global pooling via fp8 DoubleRow matmuls
            # Each sample streams as two w-halves (separate tiles) so its
            # first 14 matmuls overlap the second half's DMA.
            HQ = NQ // 2
            folds = []
            prev_fth = None
            for b in range(BS):
                acc = pp.tile([3, 2 * C], f32, tag="acc")
                # first/last sample stream in quarters: sample 0 so the
                # first matmul isn't gated on a full half-sample DMA, sample
                # 7 so only a quarter's matmuls trail the last DMA packet
                nsub = 2 if b in (0, BS - 1) else 1
                for half in range(2):
                    fth = ftp.tile([H, W // 2, C], fp8, tag=f"ft{half}")
                    for su in range(nsub):
                        nc.sync.dma_start(
                            fth[:, su * (56 // nsub):(su + 1) * (56 // nsub), :],
                            feat[:, b, half * 56 + su * (56 // nsub):
                                 half * 56 + (su + 1) * (56 // nsub), :])
                    # [h, q, p, wo, c]: w_local = 4q + 2wo + p; dim p is the
                    # DoubleRow contract pair, (wo, c) the 512 moving columns.
                    mv = fth[:].rearrange("h (q wo p) c -> h q p wo c",
                                          wo=2, p=2)
                    for q in range(HQ):
                        qg = half * HQ + q
                        nc.tensor.matmul(
                            acc[:],
                            wm_sb[:, b, qg, :, 0:3],
                            mv[:, q],
                            start=(qg == 0),
                            stop=(qg == NQ - 1),
                            perf_mode=DR,
                        )
                    prev_fth = fth
                # acc = [3 rows, (wo, c)]: row0 global, row1 wo=0 masked,
                # row2 wo=1 masked.  PSUM->SBUF copy with the per-row scale
                # (1/(H*W) or 1/area_b) fused in.
                sall = stp.tile([3, 2 * C], f32, tag="sall")
                nc.scalar.activation(sall[:], acc[:], Ident,
                                     scale=psc3_sb[0:3, b:b + 1])
                sall_v = sall[:].rearrange("p (wo c) -> p wo c", wo=2)
                sg = stp.tile([1, C], f32, tag="sg")
                nc.vector.tensor_add(sg[0:1, :], sall_v[0:1, 0, :],
                                     sall_v[0:1, 1, :])
                # rows 1,2 -> partition 0 (DMA crosses partitions)
                rowp = stp.tile([1, 4 * C], f32, tag="rowp")
                nc.scalar.dma_start(rowp[:], sall[1:3, :])
                rowp_v = rowp[:].rearrange("p (m wo c) -> p m wo c",
                                           m=2, wo=2)
                sm = stp.tile([1, C], f32, tag="sm")
                nc.vector.tensor_add(sm[0:1, :], rowp_v[0:1, 0, 0, :],
                                     rowp_v[0:1, 1, 1, :])
                # transpose into CT columns (pooled -> cols k*8+b, global ->
                # 16+k*8+b), delayed one sample so the PE queue never stalls
                # on a fold still in flight.
                if b == 0:
                    ctp = mpp.tile([128, 48], f32, tag="ctp")
                    # big consts ride behind sample 0's fold in FIFO order
                    nc.scalar.dma_start(w1t_sb[:], w1t[:])
                    nc.scalar.dma_start(w2t_sb[:], w2t[:])
                    nc.scalar.dma_start(w3t_sb[:], w3t[:])
                    nc.scalar.dma_start(b1_sb[:], b1[:])
                    nc.scalar.dma_start(b2_sb[:], b2[:])
                    nc.scalar.dma_start(b3_sb[:], b3[:])
                    nc.scalar.dma_start(lt[:], lang[:])
                if b == 1:
                    # lang transposes + their CT cast ride mid-stream (PE
                    # and DVE have slack); only pooled/global cols remain
                    # on the tail.
                    for k in range(2):
                        nc.tensor.transpose(
                            ctp[:, 32 + k * 8:32 + k * 8 + 8],
                            lt[:, k * 128:(k + 1) * 128],
                            id_sb[0:BS, 0:BS])
                    nc.vector.tensor_copy(ct[:, 32:48], ctp[:, 32:48])
                folds.append((b, sm, sg))
                for fb, fsm, fsg in folds[-2:-1] if b < BS - 1 else folds[-2:]:
                    for k in range(2):
                        nc.tensor.transpose(
                            ctp[:, k * 8 + fb:k * 8 + fb + 1],
                            fsm[0:1, k * CH:(k + 1) * CH], id_sb[0:1, 0:1])
                        nc.tensor.transpose(
                            ctp[:, 16 + k * 8 + fb:16 + k * 8 + fb + 1],
                            fsg[0:1, k * CH:(k + 1) * CH], id_sb[0:1, 0:1])

            nc.vector.tensor_copy(ct[:, 0:32], ctp[:, 0:32])

            rhs_k = [ct[:, 8 * k:8 * k + 8] for k in range(6)]

            # ---- layer 1: 768 -> 256, relu
            h1 = []
            for m2 in range(2):
                hp = mpp.tile([128, BS], f32, tag="h1p")
                for k in range(6):
                    nc.tensor.matmul(
                        hp[:],
                        w1t_sb[:, k * HID + m2 * 128:k * HID + m2 * 128 + 128],
                        rhs_k[k],
                        start=(k == 0), stop=(k == 5))
                ht = sp.tile([128, BS], bf16, tag=f"h1_{m2}")
                nc.scalar.activation(ht[:], hp[:], Relu,
                                     bias=b1_sb[:, m2:m2 + 1])
                h1.append(ht)

            # ---- layer 2: 256 -> 256, relu
            h2 = []
            for m2 in range(2):
                hp = mpp.tile([128, BS], f32, tag="h2p")
                for kc in range(2):
                    nc.tensor.matmul(
                        hp[:],
                        w2t_sb[:, (kc * 2 + m2) * 128:(kc * 2 + m2) * 128 + 128],
                        h1[kc][:],
                        start=(kc == 0), stop=(kc == 1))
                ht = sp.tile([128, BS], bf16, tag=f"h2_{m2}")
                nc.scalar.activation(ht[:], hp[:], Relu,
                                     bias=b2_sb[:, m2:m2 + 1])
                h2.append(ht)

            # ---- layer 3: 256 -> 1, sigmoid
            s3 = mpp.tile([1, BS], f32, tag="s3")
            for kc in range(2):
                nc.tensor.matmul(s3[:], w3t_sb[:, kc:kc + 1], h2[kc][:],
                                 start=(kc == 0), stop=(kc == 1))
            res = sp.tile([1, BS], f32, tag="res")
            nc.scalar.activation(res[:], s3[:], Sigmoid, bias=b3_sb[0:1, 0:1])
            nc.sync.dma_start(out[:], res[:])

    nc.compile()
    return nc


# ----------------------------------------------------------------- entry
def _prepare_in_maps(feat, lang_vec, boxes_xywh, w1, b1, w2, b2, w3, b3):
    import ml_dtypes

    row, col, area = _host_masks(boxes_xywh)

    w1t_arr = np.ascontiguousarray(
        w1.astype(np.float32).T.reshape(6, 128, HID)
        .transpose(1, 0, 2).reshape(128, 6 * HID)).astype(ml_dtypes.bfloat16)
    w2t_arr = np.ascontiguousarray(
        w2.astype(np.float32).T.reshape(2, 128, 2, 128)
        .transpose(1, 0, 2, 3).reshape(128, 4 * 128)).astype(ml_dtypes.bfloat16)
    w3t_arr = np.ascontiguousarray(
        w3.astype(np.float32).T.reshape(2, 128).T
        ).astype(ml_dtypes.bfloat16)                        # [128, 2]
    b1_arr = np.ascontiguousarray(b1.astype(np.float32).reshape(2, 128).T)
    b2_arr = np.ascontiguousarray(b2.astype(np.float32).reshape(2, 128).T)
    b3_arr = b3.astype(np.float32).reshape(1, 1)

    feat = feat.astype(np.float32)
    lang_vec = np.ascontiguousarray(lang_vec.astype(np.float32))

    in_maps = []
    for i in range(N_CORES):
        s = slice(i * BS, (i + 1) * BS)
        wm = _build_wm(row[s], col[s])
        in_maps.append({
            # [h, b, w, c] fp8 staging (see module docstring)
            "feat": feat[s].transpose(2, 0, 3, 1)
                    .astype(ml_dtypes.float8_e4m3),
            "wm": np.ascontiguousarray(wm),
            "psc3": np.stack([np.full(BS, 1.0 / (H * W), np.float32),
                              (1.0 / area[s]).astype(np.float32),
                              (1.0 / area[s]).astype(np.float32)]),
            "lang": lang_vec[s],
            "ident": np.eye(32, dtype=np.float32),
            "w1t": w1t_arr, "w2t": w2t_arr, "w3t": w3t_arr,
            "b1": b1_arr, "b2": b2_arr, "b3": b3_arr,
        })
    return in_maps


def kernel(feat, lang_vec, boxes_xywh, w1, b1, w2, b2, w3, b3,
           _trace=False):
    from concourse.bass_utils import run_bass_kernel_spmd

    if "nc" not in _CACHE:
        _CACHE["nc"] = _build_nc()
    nc = _CACHE["nc"]

    args = [np.asarray(a) for a in
            (feat, lang_vec, boxes_xywh, w1, b1, w2, b2, w3, b3)]
    in_maps = _prepare_in_maps(*args)
    res = None
    for attempt in range(2):
        try:
            res = run_bass_kernel_spmd(nc, in_maps,
                                       core_ids=list(range(N_CORES)),
                                       trace=_trace)
            break
        except Exception:
            if attempt == 1:
                raise
    out = np.concatenate([res.results[i]["out"].reshape(BS, 1)
                          for i in range(N_CORES)], axis=0)
    _CACHE["last_exec_time_ns"] = res.exec_time_ns
    return out.astype(np.float32)
